# revision 24
# baseline (speedup 1.0000x reference)
"""CrossAttention Trainium2 kernel (8 NeuronCores, SPMD), bf16 compute.

Sharding: data-parallel over batch B=2, tensor-parallel over the 16 heads in
4 groups of 4 heads -> 8 cores, one (batch, head-group) pair each. Each core
computes its 4 heads' Q/K/V projections, masked softmax cross-attention, and
its partial output projection; the host sums the 4 partial outputs per batch
(the Wo row-split all-reduce, done at unshard time) and adds the constant
Wo @ b_v term (softmax rows sum to 1 so it factors out of the attention).

Numerics: bf16 matmuls with fp32 PSUM accumulation; softmax statistics stay
fp32. x and the 0/1 mask travel as fp8e4m3 to halve their DMA footprint
(mask values are exact in fp8; x quantization only perturbs softmax weights,
damped by the small score scale). exp() is unnormalized (|scores*scale| <
~2.5) and runs on ACT straight out of PSUM with the softmax scale fused; the
mask is applied multiplicatively afterwards on DVE. y is written bf16; the
host accumulates partials in fp32.

Layout: activations and weights arrive contraction-major (host
pre-transposed) so every DMA is a contiguous row load; no device transposes.
Attention is scores-transposed: ST[m, n] per head, so PV contracts over m
directly. The softmax denominator comes free from an appended ones-column on
the v stationary operand. Masked exps live in rotating buffers: PV consumes
them one m-tile behind the exp; nothing is parked in SBUF.

Schedule: two n-half passes over the 16 m-tiles. Per-head single-bank score
tiles with a 3-deep rotation keep the PE fed; the V projection and the last
3/4 of the K projection ride inside pass 0 (K shares the score-tile
rotation). Q and K chunk 0 are emitted contraction-chunk-outer so the PE
consumes DMA chunks as they land. Pass 0's normalize + output projection +
full-row y writeback overlap pass 1; pass 1's tail normalizes straight from
PSUM (no park) with the scores pool closed to give the tail three PSUM
banks. Odd-head normalized outputs reach partitions 64:128 via an
identity-stationary PE matmul at column offset 64.
"""

import numpy as np
import ml_dtypes

import concourse.bass as bass
import concourse.bacc as bacc
import concourse.mybir as mybir
import concourse.tile as tile
from concourse.bass_utils import run_bass_kernel_spmd

DIM = 1024
HEAD_DIM = 64
NUM_HEADS = 16
SCALE = HEAD_DIM**-0.5
B, N, M = 2, 1024, 2048
HPC = 4  # heads per core
E = HPC * HEAD_DIM  # 256: per-core projection width
P = 128
F32 = mybir.dt.float32
BF16 = mybir.dt.bfloat16
F8 = mybir.dt.float8e4
CT = DIM // P  # 8 contraction tiles
MT = M // P  # 16 m tiles


def _bc_heads(ap, n):
    """Broadcast a [P, F] AP to [P, n, F] with a zero-stride middle dim."""
    return bass.AP(ap.tensor, ap.offset, [ap.ap[0], [0, n], ap.ap[1]])


def _group_heads(ap, hpc, hd):
    """View a [P, hpc*hd] AP as [P, hpc, hd]."""
    assert ap.ap[-1][0] == 1 and ap.ap[-1][1] == hpc * hd
    return bass.AP(ap.tensor, ap.offset, [ap.ap[0], [hd, hpc], [1, hd]])


def build_program():
    nc = bacc.Bacc("TRN2", target_bir_lowering=False, debug=False, num_devices=8)

    # contraction-major inputs (host pre-transposed)
    xT_d = nc.dram_tensor("xT", [DIM, N], F8, kind="ExternalInput").ap()
    ctxT_d = nc.dram_tensor("ctxT", [DIM, M], BF16, kind="ExternalInput").ap()
    mk_d = nc.dram_tensor("mk", [M, N], BF16, kind="ExternalInput").ap()
    wqT_d = nc.dram_tensor("wqT", [DIM, E], BF16, kind="ExternalInput").ap()
    wkT_d = nc.dram_tensor("wkT", [DIM, E], BF16, kind="ExternalInput").ap()
    wvT_d = nc.dram_tensor("wvT", [DIM, E], BF16, kind="ExternalInput").ap()
    woT_d = nc.dram_tensor("woT", [E, DIM], BF16, kind="ExternalInput").ap()
    bk_d = nc.dram_tensor("bk", [E], F32, kind="ExternalInput").ap()
    eye_d = nc.dram_tensor(
        "eye64", [HEAD_DIM, HEAD_DIM], BF16, kind="ExternalInput"
    ).ap()
    y_d = nc.dram_tensor("y", [N, DIM], BF16, kind="ExternalOutput").ap()

    Exp = mybir.ActivationFunctionType.Exp

    from contextlib import ExitStack

    with tile.TileContext(nc) as tc, ExitStack() as ctx:
        const = ctx.enter_context(tc.tile_pool(name="const", bufs=1))
        bk_sb = const.tile([P, E // P], F32)
        eye64 = const.tile([HEAD_DIM, HEAD_DIM], BF16)
        ones65 = const.tile([HEAD_DIM + 1, HEAD_DIM], F32)
        nc.gpsimd.dma_start(out=bk_sb, in_=bk_d.rearrange("(t p) -> p t", p=P))
        nc.gpsimd.dma_start(out=eye64, in_=eye_d)
        nc.vector.memset(ones65, 1.0)

        persist = ctx.enter_context(tc.tile_pool(name="persist", bufs=1))
        qT = persist.tile([P, E // P, N], BF16)
        kT = persist.tile([P, E // P, M], BF16)
        vaug = persist.tile([P, MT, HPC, HEAD_DIM + 1], BF16)
        woT = persist.tile([P, E // P, DIM], BF16)
        mask = persist.tile([P, MT, N], BF16)
        ot_sb = persist.tile([HEAD_DIM + 1, HPC, N], F32)
        otn2 = persist.tile([P, E // P, N], BF16)  # normalized attn out

        # ones column for the softmax denominator; v evictions fill 0:64
        nc.vector.memset(vaug, 1.0)

        expl = ctx.enter_context(tc.tile_pool(name="expl", bufs=3))
        exml = ctx.enter_context(tc.tile_pool(name="exml", bufs=4))
        dnp = ctx.enter_context(tc.tile_pool(name="dnp", bufs=1))
        rbp = ctx.enter_context(tc.tile_pool(name="rbp", bufs=1))
        ypool = ctx.enter_context(tc.tile_pool(name="ypool", bufs=2))

        ex_tiles = {}

        def emit_scores(spool, mt, chn, hp):
            """Per-head bf16 scores -> ACT exp (PSUM read, scale fused)."""
            for hl in range(2):
                h = hp * 2 + hl
                st = spool.tile([P, 512], F32, tag="st", name="st", bufs=3)
                dr = slice(hl * HEAD_DIM, (hl + 1) * HEAD_DIM)
                nc.tensor.matmul(
                    st,
                    lhsT=kT[dr, hp, mt * P : (mt + 1) * P],
                    rhs=qT[dr, hp, chn * 512 : (chn + 1) * 512],
                    start=True,
                    stop=True,
                )
                ex = ex_tiles[(mt, chn)]
                nc.scalar.activation(ex[:, h, :], st, Exp, scale=float(SCALE))

        def emit_mask_mul(mt, chn):
            """One DVE multiply masks all 4 heads of (mt, chn)."""
            ex = ex_tiles[(mt, chn)]
            exm = exml.tile([P, HPC, 512], BF16, tag="exm", name="exm")
            mks = _bc_heads(mask[:, mt, chn * 512 : (chn + 1) * 512], HPC)
            nc.vector.tensor_mul(exm, ex, mks)
            ex_tiles[(mt, chn)] = exm  # PV reads the masked version

        def emit_pv(ot_ps, mt, chn):
            exm = ex_tiles.pop((mt, chn))
            for h in range(HPC):
                nc.tensor.matmul(
                    ot_ps[h],
                    lhsT=vaug[:, mt, h, :],
                    rhs=exm[:, h, :],
                    start=(mt == 0),
                    stop=(mt == MT - 1),
                )

        def emit_step(spool, ot_ps, mt, chn):
            """scores+exp for all heads of (mt,chn), mask-mul, PV(mt-2).
            The 2-step PV lag keeps the exp->mask-mul latency off the PE's
            in-order critical path."""
            ex_tiles[(mt, chn)] = expl.tile([P, HPC, 512], BF16, tag="ex", name="ex")
            emit_scores(spool, mt, chn, 0)
            yield  # pass-specific PE filler slot (V proj / K proj / O proj)
            emit_scores(spool, mt, chn, 1)
            emit_mask_mul(mt, chn)
            if mt > 1:
                emit_pv(ot_ps, mt - 2, chn)
            yield

        # ---------------- input DMAs ---------------------------------------
        with tc.tile_pool(name="wx", bufs=1) as wx_pool:
            wqT = wx_pool.tile([P, CT, E], BF16)
            xT = wx_pool.tile([P, CT, N], F8)
            wkT = wx_pool.tile([P, CT, E], BF16)
            wvT = wx_pool.tile([P, CT, E], BF16)
            ctxT = wx_pool.tile([P, CT, M], BF16)

            # DMA rings are issue-rate bound (~0.6us/instruction), so chunks
            # are as big as dependency granularity allows.
            # sync ring: wq, x (Q path, 2-j chunks), mask (4-mt chunks), wo
            nc.sync.dma_start(out=wqT, in_=wqT_d.rearrange("(c p) e -> p c e", p=P))
            for jp in range(CT // 2):
                nc.sync.dma_start(
                    out=xT[:, 2 * jp : 2 * jp + 2, :],
                    in_=xT_d[jp * 2 * P : (jp + 1) * 2 * P, :].rearrange(
                        "(c p) n -> p c n", p=P
                    ),
                )
            for mq in range(4):
                nc.sync.dma_start(
                    out=mask[:, 4 * mq : 4 * mq + 4, :],
                    in_=mk_d[mq * 4 * P : (mq + 1) * 4 * P, :].rearrange(
                        "(c p) n -> p c n", p=P
                    ),
                )
            nc.sync.dma_start(out=woT, in_=woT_d.rearrange("(c p) e -> p c e", p=P))
            # scalar ring: wk, ctx q0 fine-grained (K chunk 0 gate), q1-q2
            nc.scalar.dma_start(out=wkT, in_=wkT_d.rearrange("(c p) e -> p c e", p=P))
            for j in range(CT):
                nc.scalar.dma_start(
                    out=ctxT[:, j, :512], in_=ctxT_d[j * P : (j + 1) * P, :512]
                )
            for q in range(1, 3):
                for jp in range(CT // 2):
                    nc.scalar.dma_start(
                        out=ctxT[:, 2 * jp : 2 * jp + 2, q * 512 : (q + 1) * 512],
                        in_=ctxT_d[
                            jp * 2 * P : (jp + 1) * 2 * P, q * 512 : (q + 1) * 512
                        ].rearrange("(c p) m -> p c m", p=P),
                    )
            # gpsimd ring: wv only up front; ctx q3 rides inside pass 0
            nc.gpsimd.dma_start(out=wvT, in_=wvT_d.rearrange("(c p) e -> p c e", p=P))

            def emit_ctx_q3(jp):
                nc.gpsimd.dma_start(
                    out=ctxT[:, 2 * jp : 2 * jp + 2, 1536:],
                    in_=ctxT_d[jp * 2 * P : (jp + 1) * 2 * P, 1536:].rearrange(
                        "(c p) m -> p c m", p=P
                    ),
                )

            # ---------------- head projections + pass 0 --------------------
            with (
                tc.tile_pool(name="sps0", bufs=1, space="PSUM") as sps0,
                tc.tile_pool(name="vps", bufs=1, space="PSUM") as vps,
            ):

                def emit_kproj(et, chm):
                    # shares the score-tile single-bank rotation (tag "st");
                    # eviction rides the otherwise-idle gpsimd engine
                    pk = sps0.tile([P, 512], F32, tag="st", name="pk", bufs=3)
                    for j in range(CT):
                        nc.tensor.matmul(
                            pk,
                            lhsT=wkT[:, j, et * P : (et + 1) * P],
                            rhs=ctxT[:, j, chm * 512 : (chm + 1) * 512],
                            start=(j == 0),
                            stop=(j == CT - 1),
                        )
                    nc.scalar.add(
                        kT[:, et, chm * 512 : (chm + 1) * 512],
                        pk,
                        bk_sb[:, et : et + 1],
                    )

                def emit_kproj0():
                    # both et groups of chunk 0, contraction-chunk outer
                    pks = [sps0.tile([P, 512], F32, tag="st", name="pk", bufs=3)
                           for _ in range(2)]
                    for j in range(CT):
                        for et in range(2):
                            nc.tensor.matmul(
                                pks[et],
                                lhsT=wkT[:, j, et * P : (et + 1) * P],
                                rhs=ctxT[:, j, :512],
                                start=(j == 0),
                                stop=(j == CT - 1),
                            )
                    for et in range(2):
                        nc.vector.tensor_scalar_add(
                            kT[:, et, :512], pks[et], bk_sb[:, et : et + 1]
                        )

                def emit_vproj(mt):
                    pv = vps.tile([P, E], F32, tag="pv", name="pv")
                    for j in range(CT):
                        nc.tensor.matmul(
                            pv,
                            lhsT=ctxT[:, j, mt * P : (mt + 1) * P],
                            rhs=wvT[:, j, :],
                            start=(j == 0),
                            stop=(j == CT - 1),
                        )
                    nc.scalar.copy(
                        vaug[:, mt, :, :HEAD_DIM],
                        _group_heads(pv[:, :], HPC, HEAD_DIM),
                    )

                # head: K chunk 0 and V(0) first (ctx q0 lands before x),
                # then the x-gated Q projection, then V(1-3) and K chunk 1
                emit_kproj0()
                emit_vproj(0)
                qgroups = [
                    (et, chn) for et in range(E // P) for chn in range(N // 512)
                ]
                with tc.tile_pool(name="qps", bufs=1, space="PSUM") as qps:
                    pqs = {g: qps.tile([P, 512], F32, tag=f"pq{i}", name=f"pq{i}")
                           for i, g in enumerate(qgroups)}
                    for j in range(CT):
                        for et, chn in qgroups:
                            nc.tensor.matmul(
                                pqs[(et, chn)],
                                lhsT=wqT[:, j, et * P : (et + 1) * P],
                                rhs=xT[:, j, chn * 512 : (chn + 1) * 512],
                                start=(j == 0),
                                stop=(j == CT - 1),
                            )
                    for et, chn in qgroups:
                        nc.vector.tensor_copy(
                            qT[:, et, chn * 512 : (chn + 1) * 512], pqs[(et, chn)]
                        )
                for mt in range(1, 4):
                    emit_vproj(mt)
                emit_kproj(0, 1)
                emit_kproj(1, 1)

                kfill = {2: (0, 2), 3: (1, 2), 8: (0, 3), 9: (1, 3)}
                with tc.tile_pool(name="ops0", bufs=1, space="PSUM") as ops0:
                    ot_ps0 = [
                        ops0.tile([HEAD_DIM + 1, 512], F32, tag=f"o{h}", name=f"o{h}")
                        for h in range(HPC)
                    ]
                    for mt in range(MT):
                        step = emit_step(sps0, ot_ps0, mt, 0)
                        next(step)
                        if 3 <= mt < MT - 1:
                            emit_vproj(mt + 1)
                        next(step, None)
                        if mt <= 3:
                            emit_ctx_q3(mt)
                        if mt in kfill:
                            emit_kproj(*kfill[mt])
                    emit_pv(ot_ps0, MT - 2, 0)
                    emit_pv(ot_ps0, MT - 1, 0)
                    for h in range(HPC):
                        nc.vector.tensor_copy(ot_sb[:, h, :512], ot_ps0[h])

        def normalize_all(chn, srcs):
            """Normalize all 4 heads' n-half chn. srcs[h] rows 0:64 hold the
            unnormalized outputs (SBUF park or PSUM accumulators); the
            denominators must already sit in ot_sb row 64. One strided
            SBUF-SBUF DMA + one reciprocal + one partition_broadcast cover
            all heads (the broadcast is per-instruction-overhead bound).
            Returns {h: tmp} for the odd heads, for normalize_shift."""
            cs = slice(chn * 512, (chn + 1) * 512)
            dna = dnp.tile([1, HPC, 512], F32, tag="dna", name="dna")
            nc.sync.dma_start(out=dna, in_=ot_sb[HEAD_DIM : HEAD_DIM + 1, :, cs])
            rca = rbp.tile([1, HPC, 512], F32, tag="rca", name="rca")
            nc.vector.reciprocal_approx_fast(out=rca, in_=dna)
            rba = rbp.tile([HEAD_DIM, HPC, 512], F32, tag="rba", name="rba")
            nc.gpsimd.partition_broadcast(rba, rca)
            tmps = {}
            for h in range(HPC):
                hp, hl = divmod(h, 2)
                if hl == 0:
                    nc.vector.tensor_mul(otn2[:HEAD_DIM, hp, cs], srcs[h], rba[:, h, :])
                else:
                    tmp = rbp.tile([HEAD_DIM, 512], BF16, tag=f"tmp{h}", name="tmp")
                    nc.vector.tensor_mul(tmp, srcs[h], rba[:, h, :])
                    tmps[h] = tmp
            return tmps

        def normalize_shift(h, chn, tmp, yps):
            """odd-head normalized out -> partitions 64:128 via identity mm."""
            cs = slice(chn * 512, (chn + 1) * 512)
            hp = h // 2
            sh = yps.tile([P, 512], F32, tag="yp", name="sh")
            nc.tensor.matmul(
                sh[HEAD_DIM:P, :], lhsT=eye64, rhs=tmp, start=True, stop=True
            )
            nc.vector.tensor_copy(otn2[HEAD_DIM:P, hp, cs], sh[HEAD_DIM:P, :])

        def emit_oproj(yps, nb, ring):
            ys = ypool.tile([P, N], BF16, tag="ys", name="ys")
            for oc in range(DIM // 512):
                yp = yps.tile([P, 512], F32, tag="yp", name="yp")
                for hp in range(E // P):
                    nc.tensor.matmul(
                        yp,
                        lhsT=otn2[:, hp, nb * P : (nb + 1) * P],
                        rhs=woT[:, hp, oc * 512 : (oc + 1) * 512],
                        start=(hp == 0),
                        stop=(hp == E // P - 1),
                    )
                nc.vector.tensor_copy(ys[:, oc * 512 : (oc + 1) * 512], yp)
            ring.dma_start(out=y_d[nb * P : (nb + 1) * P, :], in_=ys)

        # ---------------- pass 1: n-cols 512:1024 + pass-0 tail work -------
        with tc.tile_pool(name="ops1", bufs=1, space="PSUM") as ops1:
            ot_ps1 = [
                ops1.tile([HEAD_DIM + 1, 512], F32, tag=f"p{h}", name=f"p{h}")
                for h in range(HPC)
            ]
            with (
                tc.tile_pool(name="sps1", bufs=1, space="PSUM") as sps1,
                tc.tile_pool(name="yps0", bufs=1, space="PSUM") as yps0,
            ):
                tmps = {}
                for mt in range(MT):
                    step = emit_step(sps1, ot_ps1, mt, 1)
                    next(step)
                    if mt in (8, 10, 12, 14):
                        emit_oproj(yps0, (mt - 8) // 2, nc.sync)
                    next(step, None)
                    if mt == 0:
                        tmps = normalize_all(
                            0, [ot_sb[:HEAD_DIM, h, :512] for h in range(HPC)]
                        )
                    elif mt == 6:
                        for h in (1, 3):
                            normalize_shift(h, 0, tmps[h], yps0)
                emit_pv(ot_ps1, MT - 2, 1)
                emit_pv(ot_ps1, MT - 1, 1)
                # stage pass-1 denominators on partition 64 for the tail
                for h in range(HPC):
                    nc.scalar.copy(
                        ot_sb[HEAD_DIM : HEAD_DIM + 1, h, 512:],
                        ot_ps1[h][HEAD_DIM : HEAD_DIM + 1, :],
                    )

            # tail: normalize straight from PSUM (scores pool closed -> 3
            # free banks), finish O proj, full-row y writeback on 3 rings
            with tc.tile_pool(name="yps1", bufs=3, space="PSUM") as yps1:
                tmps = normalize_all(1, [ot_ps1[h][:HEAD_DIM, :] for h in range(HPC)])
                for h in (1, 3):
                    normalize_shift(h, 1, tmps[h], yps1)
                rings = [nc.sync, nc.scalar, nc.gpsimd, nc.scalar]
                for i, nb in enumerate(range(N // P // 2, N // P)):
                    emit_oproj(yps1, nb, rings[i])

    nc.compile()
    return nc


_NC_CACHE = []


def _get_nc():
    if not _NC_CACHE:
        _NC_CACHE.append(build_program())
    return _NC_CACHE[0]


def make_in_maps(x, context, mask, Wq, Wkv, b_kv, Wo):
    bf = ml_dtypes.bfloat16
    f8 = ml_dtypes.float8_e4m3
    x = np.asarray(x, dtype=np.float32)
    context = np.asarray(context, dtype=np.float32)
    mask = np.asarray(mask)
    Wq = np.asarray(Wq, dtype=np.float32)
    Wkv = np.asarray(Wkv, dtype=np.float32)
    b_kv = np.asarray(b_kv, dtype=np.float32)
    Wo = np.asarray(Wo, dtype=np.float32)
    eye = np.eye(HEAD_DIM, dtype=bf)

    in_maps = []
    for b in range(B):
        xtb = np.ascontiguousarray(x[b].T).astype(f8)
        ctb = np.ascontiguousarray(context[b].T).astype(bf)
        mkb = np.ascontiguousarray(mask[b].T).astype(bf)
        for g in range(NUM_HEADS // HPC):
            sl = slice(E * g, E * (g + 1))
            in_maps.append(
                {
                    "xT": xtb,
                    "ctxT": ctb,
                    "mk": mkb,
                    "wqT": np.ascontiguousarray(Wq[sl].T).astype(bf),
                    "wkT": np.ascontiguousarray(Wkv[sl].T).astype(bf),
                    "wvT": np.ascontiguousarray(
                        Wkv[DIM + E * g : DIM + E * (g + 1)].T
                    ).astype(bf),
                    "woT": np.ascontiguousarray(Wo[:, sl].T).astype(bf),
                    "bk": np.ascontiguousarray(b_kv[sl]),
                    "eye64": eye,
                }
            )
    return in_maps


def combine_outputs(ys, b_kv, Wo):
    """ys: list of 8 per-core partial outputs [N, DIM], core order (b, g)."""
    b_v = np.asarray(b_kv, dtype=np.float32)[DIM:]
    ybias = np.asarray(Wo, dtype=np.float32) @ b_v  # [DIM]
    out = np.empty((B, N, DIM), dtype=np.float32)
    G = NUM_HEADS // HPC
    for b in range(B):
        acc = np.asarray(ys[G * b], dtype=np.float32)
        for g in range(1, G):
            acc = acc + np.asarray(ys[G * b + g], dtype=np.float32)
        out[b] = acc + ybias[None, :]
    return out


def kernel(x, context, mask, Wq, Wkv, b_kv, Wo):
    nc = _get_nc()
    in_maps = make_in_maps(x, context, mask, Wq, Wkv, b_kv, Wo)
    res = run_bass_kernel_spmd(nc, in_maps, core_ids=list(range(8)))
    ys = [m["y"] for m in res.results]
    return combine_outputs(ys, b_kv, Wo)


# revision 25
# speedup vs baseline: 1.0468x; 1.0468x over previous
"""CrossAttention Trainium2 kernel (8 NeuronCores, SPMD), bf16 compute.

Sharding: data-parallel over batch B=2, tensor-parallel over the 16 heads in
4 groups of 4 heads -> 8 cores, one (batch, head-group) pair each. Each core
computes its 4 heads' Q/K/V projections, masked softmax cross-attention, and
its partial output projection; the host sums the 4 partial outputs per batch
(the Wo row-split all-reduce, done at unshard time) and adds the constant
Wo @ b_v term (softmax rows sum to 1 so it factors out of the attention).

Numerics: bf16 matmuls with fp32 PSUM accumulation; softmax statistics stay
fp32. x and the 0/1 mask travel as fp8e4m3 to halve their DMA footprint
(mask values are exact in fp8; x quantization only perturbs softmax weights,
damped by the small score scale). exp() is unnormalized (|scores*scale| <
~2.5) and runs on ACT straight out of PSUM with the softmax scale fused; the
mask is applied multiplicatively afterwards on DVE. y is written bf16; the
host accumulates partials in fp32.

Layout: activations and weights arrive contraction-major (host
pre-transposed) so every DMA is a contiguous row load; no device transposes.
Attention is scores-transposed: ST[m, n] per head, so PV contracts over m
directly. The softmax denominator comes free from an appended ones-column on
the v stationary operand. Masked exps live in rotating buffers: PV consumes
them one m-tile behind the exp; nothing is parked in SBUF.

Schedule: two n-half passes over the 16 m-tiles. Per-head single-bank score
tiles with a 3-deep rotation keep the PE fed; the V projection and the last
3/4 of the K projection ride inside pass 0 (K shares the score-tile
rotation). Q and K chunk 0 are emitted contraction-chunk-outer so the PE
consumes DMA chunks as they land. Pass 0's normalize + output projection +
full-row y writeback overlap pass 1; pass 1's tail normalizes straight from
PSUM (no park) with the scores pool closed to give the tail three PSUM
banks. Odd-head normalized outputs reach partitions 64:128 via an
identity-stationary PE matmul at column offset 64.
"""

import numpy as np
import ml_dtypes

import concourse.bass as bass
import concourse.bacc as bacc
import concourse.mybir as mybir
import concourse.tile as tile
from concourse.bass_utils import run_bass_kernel_spmd

DIM = 1024
HEAD_DIM = 64
NUM_HEADS = 16
SCALE = HEAD_DIM**-0.5
B, N, M = 2, 1024, 2048
HPC = 4  # heads per core
E = HPC * HEAD_DIM  # 256: per-core projection width
P = 128
F32 = mybir.dt.float32
BF16 = mybir.dt.bfloat16
F8 = mybir.dt.float8e4
CT = DIM // P  # 8 contraction tiles
MT = M // P  # 16 m tiles


def _bc_heads(ap, n):
    """Broadcast a [P, F] AP to [P, n, F] with a zero-stride middle dim."""
    return bass.AP(ap.tensor, ap.offset, [ap.ap[0], [0, n], ap.ap[1]])


def _group_heads(ap, hpc, hd):
    """View a [P, hpc*hd] AP as [P, hpc, hd]."""
    assert ap.ap[-1][0] == 1 and ap.ap[-1][1] == hpc * hd
    return bass.AP(ap.tensor, ap.offset, [ap.ap[0], [hd, hpc], [1, hd]])


def build_program():
    nc = bacc.Bacc("TRN2", target_bir_lowering=False, debug=False, num_devices=8)

    # contraction-major inputs (host pre-transposed)
    xT_d = nc.dram_tensor("xT", [DIM, N], F8, kind="ExternalInput").ap()
    ctxT_d = nc.dram_tensor("ctxT", [DIM, M], BF16, kind="ExternalInput").ap()
    mk_d = nc.dram_tensor("mk", [M, N], BF16, kind="ExternalInput").ap()
    wqT_d = nc.dram_tensor("wqT", [DIM, E], BF16, kind="ExternalInput").ap()
    wkT_d = nc.dram_tensor("wkT", [DIM, E], BF16, kind="ExternalInput").ap()
    wvT_d = nc.dram_tensor("wvT", [DIM, E], BF16, kind="ExternalInput").ap()
    woT_d = nc.dram_tensor("woT", [E, DIM], BF16, kind="ExternalInput").ap()
    bk_d = nc.dram_tensor("bk", [E], F32, kind="ExternalInput").ap()
    eye_d = nc.dram_tensor(
        "eye64", [HEAD_DIM, HEAD_DIM], BF16, kind="ExternalInput"
    ).ap()
    y_d = nc.dram_tensor("y", [N, DIM], BF16, kind="ExternalOutput").ap()

    Exp = mybir.ActivationFunctionType.Exp

    from contextlib import ExitStack

    with tile.TileContext(nc) as tc, ExitStack() as ctx:
        const = ctx.enter_context(tc.tile_pool(name="const", bufs=1))
        bk_sb = const.tile([P, E // P], F32)
        eye64 = const.tile([HEAD_DIM, HEAD_DIM], BF16)
        ones65 = const.tile([HEAD_DIM + 1, HEAD_DIM], F32)
        nc.gpsimd.dma_start(out=bk_sb, in_=bk_d.rearrange("(t p) -> p t", p=P))
        nc.gpsimd.dma_start(out=eye64, in_=eye_d)
        nc.vector.memset(ones65, 1.0)

        persist = ctx.enter_context(tc.tile_pool(name="persist", bufs=1))
        qT = persist.tile([P, E // P, N], BF16)
        kT = persist.tile([P, E // P, M], BF16)
        vaug = persist.tile([P, MT, HPC, HEAD_DIM + 1], BF16)
        woT = persist.tile([P, E // P, DIM], BF16)
        mask = persist.tile([P, MT, N], BF16)
        ot_sb = persist.tile([HEAD_DIM + 1, HPC, N], F32)
        otn2 = persist.tile([P, E // P, N], BF16)  # normalized attn out

        # ones column for the softmax denominator; v evictions fill 0:64
        nc.vector.memset(vaug, 1.0)

        expl = ctx.enter_context(tc.tile_pool(name="expl", bufs=3))
        exml = ctx.enter_context(tc.tile_pool(name="exml", bufs=4))
        dnp = ctx.enter_context(tc.tile_pool(name="dnp", bufs=1))
        rbp = ctx.enter_context(tc.tile_pool(name="rbp", bufs=1))
        ypool = ctx.enter_context(tc.tile_pool(name="ypool", bufs=2))

        ex_tiles = {}

        def emit_scores(spool, mt, chn, hp):
            """Per-head bf16 scores -> ACT exp (PSUM read, scale fused)."""
            for hl in range(2):
                h = hp * 2 + hl
                st = spool.tile([P, 512], F32, tag="st", name="st", bufs=3)
                dr = slice(hl * HEAD_DIM, (hl + 1) * HEAD_DIM)
                nc.tensor.matmul(
                    st,
                    lhsT=kT[dr, hp, mt * P : (mt + 1) * P],
                    rhs=qT[dr, hp, chn * 512 : (chn + 1) * 512],
                    start=True,
                    stop=True,
                )
                ex = ex_tiles[(mt, chn)]
                nc.scalar.activation(ex[:, h, :], st, Exp, scale=float(SCALE))

        def emit_mask_mul(mt, chn):
            """One DVE multiply masks all 4 heads of (mt, chn)."""
            ex = ex_tiles[(mt, chn)]
            exm = exml.tile([P, HPC, 512], BF16, tag="exm", name="exm")
            mks = _bc_heads(mask[:, mt, chn * 512 : (chn + 1) * 512], HPC)
            nc.vector.tensor_mul(exm, ex, mks)
            ex_tiles[(mt, chn)] = exm  # PV reads the masked version

        def emit_pv(ot_ps, mt, chn):
            exm = ex_tiles.pop((mt, chn))
            for h in range(HPC):
                nc.tensor.matmul(
                    ot_ps[h],
                    lhsT=vaug[:, mt, h, :],
                    rhs=exm[:, h, :],
                    start=(mt == 0),
                    stop=(mt == MT - 1),
                )

        def emit_step(spool, ot_ps, mt, chn):
            """scores+exp for all heads of (mt,chn), mask-mul, PV(mt-2).
            The 2-step PV lag keeps the exp->mask-mul latency off the PE's
            in-order critical path."""
            ex_tiles[(mt, chn)] = expl.tile([P, HPC, 512], BF16, tag="ex", name="ex")
            emit_scores(spool, mt, chn, 0)
            yield  # pass-specific PE filler slot (V proj / K proj / O proj)
            emit_scores(spool, mt, chn, 1)
            emit_mask_mul(mt, chn)
            if mt > 1:
                emit_pv(ot_ps, mt - 2, chn)
            yield

        # ---------------- input DMAs ---------------------------------------
        with tc.tile_pool(name="wx", bufs=1) as wx_pool:
            wqT = wx_pool.tile([P, CT, E], BF16)
            xT = wx_pool.tile([P, CT, N], F8)
            wkT = wx_pool.tile([P, CT, E], BF16)
            wvT = wx_pool.tile([P, CT, E], BF16)
            ctxT = wx_pool.tile([P, CT, M], BF16)

            # DMA rings are issue-rate bound (~0.6us/instruction), so chunks
            # are as big as dependency granularity allows.
            # sync ring: wq, x (Q path, 2-j chunks), mask (4-mt chunks), wo
            nc.sync.dma_start(out=wqT, in_=wqT_d.rearrange("(c p) e -> p c e", p=P))
            for jp in range(CT // 2):
                nc.sync.dma_start(
                    out=xT[:, 2 * jp : 2 * jp + 2, :],
                    in_=xT_d[jp * 2 * P : (jp + 1) * 2 * P, :].rearrange(
                        "(c p) n -> p c n", p=P
                    ),
                )
            for mq in range(4):
                nc.sync.dma_start(
                    out=mask[:, 4 * mq : 4 * mq + 4, :],
                    in_=mk_d[mq * 4 * P : (mq + 1) * 4 * P, :].rearrange(
                        "(c p) n -> p c n", p=P
                    ),
                )
            nc.sync.dma_start(out=woT, in_=woT_d.rearrange("(c p) e -> p c e", p=P))
            # scalar ring: wk, ctx q0 fine-grained (K chunk 0 gate), q1-q2
            nc.scalar.dma_start(out=wkT, in_=wkT_d.rearrange("(c p) e -> p c e", p=P))
            for j in range(CT):
                nc.scalar.dma_start(
                    out=ctxT[:, j, :512], in_=ctxT_d[j * P : (j + 1) * P, :512]
                )
            for q in range(1, 3):
                for jp in range(CT // 2):
                    nc.scalar.dma_start(
                        out=ctxT[:, 2 * jp : 2 * jp + 2, q * 512 : (q + 1) * 512],
                        in_=ctxT_d[
                            jp * 2 * P : (jp + 1) * 2 * P, q * 512 : (q + 1) * 512
                        ].rearrange("(c p) m -> p c m", p=P),
                    )
            # gpsimd ring: wv only up front; ctx q3 rides inside pass 0
            nc.gpsimd.dma_start(out=wvT, in_=wvT_d.rearrange("(c p) e -> p c e", p=P))

            def emit_ctx_q3(jp):
                nc.gpsimd.dma_start(
                    out=ctxT[:, 2 * jp : 2 * jp + 2, 1536:],
                    in_=ctxT_d[jp * 2 * P : (jp + 1) * 2 * P, 1536:].rearrange(
                        "(c p) m -> p c m", p=P
                    ),
                )

            # ------- Q projection, contraction-chunk outer (x-gated) -------
            qgroups = [(et, chn) for et in range(E // P) for chn in range(N // 512)]
            with tc.tile_pool(name="qps", bufs=1, space="PSUM") as qps:
                pqs = {g: qps.tile([P, 512], F32, tag=f"pq{i}", name=f"pq{i}")
                       for i, g in enumerate(qgroups)}
                for j in range(CT):
                    for et, chn in qgroups:
                        nc.tensor.matmul(
                            pqs[(et, chn)],
                            lhsT=wqT[:, j, et * P : (et + 1) * P],
                            rhs=xT[:, j, chn * 512 : (chn + 1) * 512],
                            start=(j == 0),
                            stop=(j == CT - 1),
                        )
                for et, chn in qgroups:
                    nc.vector.tensor_copy(
                        qT[:, et, chn * 512 : (chn + 1) * 512], pqs[(et, chn)]
                    )

            # ---------------- pass 0 with V/K projections inline -----------
            with (
                tc.tile_pool(name="sps0", bufs=1, space="PSUM") as sps0,
                tc.tile_pool(name="vps", bufs=1, space="PSUM") as vps,
            ):

                def emit_kproj(et, chm):
                    # shares the score-tile single-bank rotation (tag "st");
                    # eviction rides the otherwise-idle gpsimd engine
                    pk = sps0.tile([P, 512], F32, tag="st", name="pk", bufs=3)
                    for j in range(CT):
                        nc.tensor.matmul(
                            pk,
                            lhsT=wkT[:, j, et * P : (et + 1) * P],
                            rhs=ctxT[:, j, chm * 512 : (chm + 1) * 512],
                            start=(j == 0),
                            stop=(j == CT - 1),
                        )
                    nc.scalar.add(
                        kT[:, et, chm * 512 : (chm + 1) * 512],
                        pk,
                        bk_sb[:, et : et + 1],
                    )

                def emit_kproj0():
                    # both et groups of chunk 0, contraction-chunk outer
                    pks = [sps0.tile([P, 512], F32, tag="st", name="pk", bufs=3)
                           for _ in range(2)]
                    for j in range(CT):
                        for et in range(2):
                            nc.tensor.matmul(
                                pks[et],
                                lhsT=wkT[:, j, et * P : (et + 1) * P],
                                rhs=ctxT[:, j, :512],
                                start=(j == 0),
                                stop=(j == CT - 1),
                            )
                    for et in range(2):
                        nc.vector.tensor_scalar_add(
                            kT[:, et, :512], pks[et], bk_sb[:, et : et + 1]
                        )

                def emit_vproj(mt):
                    pv = vps.tile([P, E], F32, tag="pv", name="pv")
                    for j in range(CT):
                        nc.tensor.matmul(
                            pv,
                            lhsT=ctxT[:, j, mt * P : (mt + 1) * P],
                            rhs=wvT[:, j, :],
                            start=(j == 0),
                            stop=(j == CT - 1),
                        )
                    nc.scalar.copy(
                        vaug[:, mt, :, :HEAD_DIM],
                        _group_heads(pv[:, :], HPC, HEAD_DIM),
                    )

                emit_kproj0()
                emit_vproj(0)

                kfill = {2: (0, 1), 3: (1, 1), 6: (0, 2), 7: (1, 2),
                         10: (0, 3), 11: (1, 3)}
                with tc.tile_pool(name="ops0", bufs=1, space="PSUM") as ops0:
                    ot_ps0 = [
                        ops0.tile([HEAD_DIM + 1, 512], F32, tag=f"o{h}", name=f"o{h}")
                        for h in range(HPC)
                    ]
                    for mt in range(MT):
                        step = emit_step(sps0, ot_ps0, mt, 0)
                        next(step)
                        if mt < MT - 1:
                            emit_vproj(mt + 1)
                        next(step, None)
                        if 2 <= mt <= 5:
                            emit_ctx_q3(mt - 2)
                        if mt in kfill:
                            emit_kproj(*kfill[mt])
                    emit_pv(ot_ps0, MT - 2, 0)
                    emit_pv(ot_ps0, MT - 1, 0)
                    for h in range(HPC):
                        nc.vector.tensor_copy(ot_sb[:, h, :512], ot_ps0[h])

        def normalize_all(chn, srcs):
            """Normalize all 4 heads' n-half chn. srcs[h] rows 0:64 hold the
            unnormalized outputs (SBUF park or PSUM accumulators); the
            denominators must already sit in ot_sb row 64. One strided
            SBUF-SBUF DMA + one reciprocal + one partition_broadcast cover
            all heads (the broadcast is per-instruction-overhead bound).
            Returns {h: tmp} for the odd heads, for normalize_shift."""
            cs = slice(chn * 512, (chn + 1) * 512)
            dna = dnp.tile([1, HPC, 512], F32, tag="dna", name="dna")
            nc.sync.dma_start(out=dna, in_=ot_sb[HEAD_DIM : HEAD_DIM + 1, :, cs])
            rca = rbp.tile([1, HPC, 512], F32, tag="rca", name="rca")
            nc.vector.reciprocal_approx_fast(out=rca, in_=dna)
            rba = rbp.tile([HEAD_DIM, HPC, 512], F32, tag="rba", name="rba")
            nc.gpsimd.partition_broadcast(rba, rca)
            tmps = {}
            for h in range(HPC):
                hp, hl = divmod(h, 2)
                if hl == 0:
                    nc.vector.tensor_mul(otn2[:HEAD_DIM, hp, cs], srcs[h], rba[:, h, :])
                else:
                    tmp = rbp.tile([HEAD_DIM, 512], BF16, tag=f"tmp{h}", name="tmp")
                    nc.vector.tensor_mul(tmp, srcs[h], rba[:, h, :])
                    tmps[h] = tmp
            return tmps

        def normalize_shift(h, chn, tmp, yps):
            """odd-head normalized out -> partitions 64:128 via identity mm."""
            cs = slice(chn * 512, (chn + 1) * 512)
            hp = h // 2
            sh = yps.tile([P, 512], F32, tag="yp", name="sh")
            nc.tensor.matmul(
                sh[HEAD_DIM:P, :], lhsT=eye64, rhs=tmp, start=True, stop=True
            )
            nc.vector.tensor_copy(otn2[HEAD_DIM:P, hp, cs], sh[HEAD_DIM:P, :])

        def emit_oproj(yps, nb, ring):
            ys = ypool.tile([P, N], BF16, tag="ys", name="ys")
            for oc in range(DIM // 512):
                yp = yps.tile([P, 512], F32, tag="yp", name="yp")
                for hp in range(E // P):
                    nc.tensor.matmul(
                        yp,
                        lhsT=otn2[:, hp, nb * P : (nb + 1) * P],
                        rhs=woT[:, hp, oc * 512 : (oc + 1) * 512],
                        start=(hp == 0),
                        stop=(hp == E // P - 1),
                    )
                nc.vector.tensor_copy(ys[:, oc * 512 : (oc + 1) * 512], yp)
            ring.dma_start(out=y_d[nb * P : (nb + 1) * P, :], in_=ys)

        # ---------------- pass 1: n-cols 512:1024 + pass-0 tail work -------
        with tc.tile_pool(name="ops1", bufs=1, space="PSUM") as ops1:
            ot_ps1 = [
                ops1.tile([HEAD_DIM + 1, 512], F32, tag=f"p{h}", name=f"p{h}")
                for h in range(HPC)
            ]
            with (
                tc.tile_pool(name="sps1", bufs=1, space="PSUM") as sps1,
                tc.tile_pool(name="yps0", bufs=1, space="PSUM") as yps0,
            ):
                tmps = {}
                for mt in range(MT):
                    step = emit_step(sps1, ot_ps1, mt, 1)
                    next(step)
                    if mt in (4, 6, 8, 10):
                        emit_oproj(yps0, (mt - 4) // 2, nc.sync)
                    next(step, None)
                    if mt == 0:
                        tmps = normalize_all(
                            0, [ot_sb[:HEAD_DIM, h, :512] for h in range(HPC)]
                        )
                    elif mt == 2:
                        for h in (1, 3):
                            normalize_shift(h, 0, tmps[h], yps0)
                emit_pv(ot_ps1, MT - 2, 1)
                emit_pv(ot_ps1, MT - 1, 1)
                # stage pass-1 denominators on partition 64 for the tail
                for h in range(HPC):
                    nc.scalar.copy(
                        ot_sb[HEAD_DIM : HEAD_DIM + 1, h, 512:],
                        ot_ps1[h][HEAD_DIM : HEAD_DIM + 1, :],
                    )

            # tail: normalize straight from PSUM (scores pool closed -> 3
            # free banks), finish O proj, full-row y writeback on 3 rings
            with tc.tile_pool(name="yps1", bufs=3, space="PSUM") as yps1:
                tmps = normalize_all(1, [ot_ps1[h][:HEAD_DIM, :] for h in range(HPC)])
                for h in (1, 3):
                    normalize_shift(h, 1, tmps[h], yps1)
                rings = [nc.sync, nc.scalar, nc.gpsimd, nc.scalar]
                for i, nb in enumerate(range(N // P // 2, N // P)):
                    emit_oproj(yps1, nb, rings[i])

    nc.compile()
    return nc


_NC_CACHE = []


def _get_nc():
    if not _NC_CACHE:
        _NC_CACHE.append(build_program())
    return _NC_CACHE[0]


def make_in_maps(x, context, mask, Wq, Wkv, b_kv, Wo):
    bf = ml_dtypes.bfloat16
    f8 = ml_dtypes.float8_e4m3
    x = np.asarray(x, dtype=np.float32)
    context = np.asarray(context, dtype=np.float32)
    mask = np.asarray(mask)
    Wq = np.asarray(Wq, dtype=np.float32)
    Wkv = np.asarray(Wkv, dtype=np.float32)
    b_kv = np.asarray(b_kv, dtype=np.float32)
    Wo = np.asarray(Wo, dtype=np.float32)
    eye = np.eye(HEAD_DIM, dtype=bf)

    in_maps = []
    for b in range(B):
        xtb = np.ascontiguousarray(x[b].T).astype(f8)
        ctb = np.ascontiguousarray(context[b].T).astype(bf)
        mkb = np.ascontiguousarray(mask[b].T).astype(bf)
        for g in range(NUM_HEADS // HPC):
            sl = slice(E * g, E * (g + 1))
            in_maps.append(
                {
                    "xT": xtb,
                    "ctxT": ctb,
                    "mk": mkb,
                    "wqT": np.ascontiguousarray(Wq[sl].T).astype(bf),
                    "wkT": np.ascontiguousarray(Wkv[sl].T).astype(bf),
                    "wvT": np.ascontiguousarray(
                        Wkv[DIM + E * g : DIM + E * (g + 1)].T
                    ).astype(bf),
                    "woT": np.ascontiguousarray(Wo[:, sl].T).astype(bf),
                    "bk": np.ascontiguousarray(b_kv[sl]),
                    "eye64": eye,
                }
            )
    return in_maps


def combine_outputs(ys, b_kv, Wo):
    """ys: list of 8 per-core partial outputs [N, DIM], core order (b, g)."""
    b_v = np.asarray(b_kv, dtype=np.float32)[DIM:]
    ybias = np.asarray(Wo, dtype=np.float32) @ b_v  # [DIM]
    out = np.empty((B, N, DIM), dtype=np.float32)
    G = NUM_HEADS // HPC
    for b in range(B):
        acc = np.asarray(ys[G * b], dtype=np.float32)
        for g in range(1, G):
            acc = acc + np.asarray(ys[G * b + g], dtype=np.float32)
        out[b] = acc + ybias[None, :]
    return out


def kernel(x, context, mask, Wq, Wkv, b_kv, Wo):
    nc = _get_nc()
    in_maps = make_in_maps(x, context, mask, Wq, Wkv, b_kv, Wo)
    res = run_bass_kernel_spmd(nc, in_maps, core_ids=list(range(8)))
    ys = [m["y"] for m in res.results]
    return combine_outputs(ys, b_kv, Wo)


# revision 26
# speedup vs baseline: 1.0853x; 1.0367x over previous
"""CrossAttention Trainium2 kernel (8 NeuronCores, SPMD), bf16 compute.

Sharding: data-parallel over batch B=2, tensor-parallel over the 16 heads in
4 groups of 4 heads -> 8 cores, one (batch, head-group) pair each. Each core
computes its 4 heads' Q/K/V projections, masked softmax cross-attention, and
its partial output projection; the host sums the 4 partial outputs per batch
(the Wo row-split all-reduce, done at unshard time) and adds the constant
Wo @ b_v term (softmax rows sum to 1 so it factors out of the attention).

Numerics: bf16 matmuls with fp32 PSUM accumulation; softmax statistics stay
fp32. x and the 0/1 mask travel as fp8e4m3 to halve their DMA footprint
(mask values are exact in fp8; x quantization only perturbs softmax weights,
damped by the small score scale). exp() is unnormalized (|scores*scale| <
~2.5) and runs on ACT straight out of PSUM with the softmax scale fused; the
mask is applied multiplicatively afterwards on DVE. y is written bf16; the
host accumulates partials in fp32.

Layout: activations and weights arrive contraction-major (host
pre-transposed) so every DMA is a contiguous row load; no device transposes.
Attention is scores-transposed: ST[m, n] per head, so PV contracts over m
directly. The softmax denominator comes free from an appended ones-column on
the v stationary operand. Masked exps live in rotating buffers: PV consumes
them one m-tile behind the exp; nothing is parked in SBUF.

Schedule: two n-half passes over the 16 m-tiles. Per-head single-bank score
tiles with a 3-deep rotation keep the PE fed; the V projection and the last
3/4 of the K projection ride inside pass 0 (K shares the score-tile
rotation). Q and K chunk 0 are emitted contraction-chunk-outer so the PE
consumes DMA chunks as they land. Pass 0's normalize + output projection +
full-row y writeback overlap pass 1; pass 1's tail normalizes straight from
PSUM (no park) with the scores pool closed to give the tail three PSUM
banks. Odd-head normalized outputs reach partitions 64:128 via an
identity-stationary PE matmul at column offset 64.
"""

import numpy as np
import ml_dtypes

import concourse.bass as bass
import concourse.bacc as bacc
import concourse.mybir as mybir
import concourse.tile as tile
from concourse.bass_utils import run_bass_kernel_spmd

DIM = 1024
HEAD_DIM = 64
NUM_HEADS = 16
SCALE = HEAD_DIM**-0.5
B, N, M = 2, 1024, 2048
HPC = 4  # heads per core
E = HPC * HEAD_DIM  # 256: per-core projection width
P = 128
F32 = mybir.dt.float32
BF16 = mybir.dt.bfloat16
F8 = mybir.dt.float8e4
CT = DIM // P  # 8 contraction tiles
MT = M // P  # 16 m tiles


def _bc_heads(ap, n):
    """Broadcast a [P, F] AP to [P, n, F] with a zero-stride middle dim."""
    return bass.AP(ap.tensor, ap.offset, [ap.ap[0], [0, n], ap.ap[1]])


def _group_heads(ap, hpc, hd):
    """View a [P, hpc*hd] AP as [P, hpc, hd]."""
    assert ap.ap[-1][0] == 1 and ap.ap[-1][1] == hpc * hd
    return bass.AP(ap.tensor, ap.offset, [ap.ap[0], [hd, hpc], [1, hd]])


def build_program():
    nc = bacc.Bacc("TRN2", target_bir_lowering=False, debug=False, num_devices=8)

    # contraction-major inputs (host pre-transposed)
    xT_d = nc.dram_tensor("xT", [DIM, N], F8, kind="ExternalInput").ap()
    ctxT_d = nc.dram_tensor("ctxT", [DIM, M], BF16, kind="ExternalInput").ap()
    mk_d = nc.dram_tensor("mk", [M, N], BF16, kind="ExternalInput").ap()
    wqT_d = nc.dram_tensor("wqT", [DIM, E], BF16, kind="ExternalInput").ap()
    wkT_d = nc.dram_tensor("wkT", [DIM, E], BF16, kind="ExternalInput").ap()
    wvT_d = nc.dram_tensor("wvT", [DIM, E], BF16, kind="ExternalInput").ap()
    woT_d = nc.dram_tensor("woT", [E, DIM], BF16, kind="ExternalInput").ap()
    bk_d = nc.dram_tensor("bk", [E], F32, kind="ExternalInput").ap()
    eye_d = nc.dram_tensor(
        "eye64", [HEAD_DIM, HEAD_DIM], BF16, kind="ExternalInput"
    ).ap()
    y_d = nc.dram_tensor("y", [N, DIM], BF16, kind="ExternalOutput").ap()

    Exp = mybir.ActivationFunctionType.Exp

    from contextlib import ExitStack

    with tile.TileContext(nc) as tc, ExitStack() as ctx:
        const = ctx.enter_context(tc.tile_pool(name="const", bufs=1))
        bk_sb = const.tile([P, E // P], F32)
        eye64 = const.tile([HEAD_DIM, HEAD_DIM], BF16)
        ones65 = const.tile([HEAD_DIM + 1, HEAD_DIM], F32)
        nc.gpsimd.dma_start(out=bk_sb, in_=bk_d.rearrange("(t p) -> p t", p=P))
        nc.gpsimd.dma_start(out=eye64, in_=eye_d)
        nc.vector.memset(ones65, 1.0)

        persist = ctx.enter_context(tc.tile_pool(name="persist", bufs=1))
        qT = persist.tile([P, E // P, N], BF16)
        kT = persist.tile([P, E // P, M], BF16)
        vaug = persist.tile([P, MT, HPC, HEAD_DIM + 1], BF16)
        woT = persist.tile([P, E // P, DIM], BF16)
        mask = persist.tile([P, MT, N], BF16)
        ot_sb = persist.tile([HEAD_DIM + 1, HPC, N], F32)
        otn2 = persist.tile([P, E // P, N], BF16)  # normalized attn out

        # ones column for the softmax denominator; v evictions fill 0:64
        nc.vector.memset(vaug, 1.0)

        expl = ctx.enter_context(tc.tile_pool(name="expl", bufs=3))
        exml = ctx.enter_context(tc.tile_pool(name="exml", bufs=4))
        dnp = ctx.enter_context(tc.tile_pool(name="dnp", bufs=1))
        rbp = ctx.enter_context(tc.tile_pool(name="rbp", bufs=1))
        ypool = ctx.enter_context(tc.tile_pool(name="ypool", bufs=2))

        ex_tiles = {}

        def emit_scores(spool, mt, chn, hp):
            """Per-head bf16 scores -> ACT exp (PSUM read, scale fused)."""
            for hl in range(2):
                h = hp * 2 + hl
                st = spool.tile([P, 512], F32, tag="st", name="st", bufs=3)
                dr = slice(hl * HEAD_DIM, (hl + 1) * HEAD_DIM)
                nc.tensor.matmul(
                    st,
                    lhsT=kT[dr, hp, mt * P : (mt + 1) * P],
                    rhs=qT[dr, hp, chn * 512 : (chn + 1) * 512],
                    start=True,
                    stop=True,
                )
                ex = ex_tiles[(mt, chn)]
                nc.scalar.activation(ex[:, h, :], st, Exp, scale=float(SCALE))

        def emit_mask_mul(mt, chn):
            """One DVE multiply masks all 4 heads of (mt, chn)."""
            ex = ex_tiles[(mt, chn)]
            exm = exml.tile([P, HPC, 512], BF16, tag="exm", name="exm")
            mks = _bc_heads(mask[:, mt, chn * 512 : (chn + 1) * 512], HPC)
            nc.vector.tensor_mul(exm, ex, mks)
            ex_tiles[(mt, chn)] = exm  # PV reads the masked version

        def emit_pv(ot_ps, mt, chn):
            exm = ex_tiles.pop((mt, chn))
            for h in range(HPC):
                nc.tensor.matmul(
                    ot_ps[h],
                    lhsT=vaug[:, mt, h, :],
                    rhs=exm[:, h, :],
                    start=(mt == 0),
                    stop=(mt == MT - 1),
                )

        def emit_step(spool, ot_ps, mt, chn):
            """scores+exp for all heads of (mt,chn), mask-mul, PV(mt-2).
            The 2-step PV lag keeps the exp->mask-mul latency off the PE's
            in-order critical path."""
            ex_tiles[(mt, chn)] = expl.tile([P, HPC, 512], BF16, tag="ex", name="ex")
            emit_scores(spool, mt, chn, 0)
            yield  # pass-specific PE filler slot (V proj / K proj / O proj)
            emit_scores(spool, mt, chn, 1)
            emit_mask_mul(mt, chn)
            if mt > 1:
                emit_pv(ot_ps, mt - 2, chn)
            yield

        # ---------------- input DMAs ---------------------------------------
        with tc.tile_pool(name="wx", bufs=1) as wx_pool:
            wqT = wx_pool.tile([P, CT, E], BF16)
            xT = wx_pool.tile([P, CT, N], F8)
            wkT = wx_pool.tile([P, CT, E], BF16)
            wvT = wx_pool.tile([P, CT, E], BF16)
            ctxT = wx_pool.tile([P, CT, M], BF16)

            # DMA rings are issue-rate bound (~0.6us/instruction), so chunks
            # are as big as dependency granularity allows.
            # sync ring: wq, x (Q path, 2-j chunks), mask (4-mt chunks), wo
            nc.sync.dma_start(out=wqT, in_=wqT_d.rearrange("(c p) e -> p c e", p=P))
            for jp in range(CT // 2):
                nc.sync.dma_start(
                    out=xT[:, 2 * jp : 2 * jp + 2, :],
                    in_=xT_d[jp * 2 * P : (jp + 1) * 2 * P, :].rearrange(
                        "(c p) n -> p c n", p=P
                    ),
                )
            for mq in range(4):
                nc.sync.dma_start(
                    out=mask[:, 4 * mq : 4 * mq + 4, :],
                    in_=mk_d[mq * 4 * P : (mq + 1) * 4 * P, :].rearrange(
                        "(c p) n -> p c n", p=P
                    ),
                )
            nc.sync.dma_start(out=woT, in_=woT_d.rearrange("(c p) e -> p c e", p=P))
            # scalar ring: wk, ctx q0 fine-grained (K chunk 0 gate), q1-q2
            nc.scalar.dma_start(out=wkT, in_=wkT_d.rearrange("(c p) e -> p c e", p=P))
            for j in range(CT):
                nc.scalar.dma_start(
                    out=ctxT[:, j, :512], in_=ctxT_d[j * P : (j + 1) * P, :512]
                )
            for q in range(1, 3):
                for jp in range(CT // 2):
                    nc.scalar.dma_start(
                        out=ctxT[:, 2 * jp : 2 * jp + 2, q * 512 : (q + 1) * 512],
                        in_=ctxT_d[
                            jp * 2 * P : (jp + 1) * 2 * P, q * 512 : (q + 1) * 512
                        ].rearrange("(c p) m -> p c m", p=P),
                    )
            # gpsimd ring: wv only up front; ctx q3 rides inside pass 0
            nc.gpsimd.dma_start(out=wvT, in_=wvT_d.rearrange("(c p) e -> p c e", p=P))

            def emit_ctx_q3(jp):
                nc.gpsimd.dma_start(
                    out=ctxT[:, 2 * jp : 2 * jp + 2, 1536:],
                    in_=ctxT_d[jp * 2 * P : (jp + 1) * 2 * P, 1536:].rearrange(
                        "(c p) m -> p c m", p=P
                    ),
                )

            # ------- Q projection, contraction-chunk outer (x-gated) -------
            qgroups = [(et, chn) for et in range(E // P) for chn in range(N // 512)]
            with tc.tile_pool(name="qps", bufs=1, space="PSUM") as qps:
                pqs = {g: qps.tile([P, 512], F32, tag=f"pq{i}", name=f"pq{i}")
                       for i, g in enumerate(qgroups)}
                for j in range(CT):
                    for et, chn in qgroups:
                        nc.tensor.matmul(
                            pqs[(et, chn)],
                            lhsT=wqT[:, j, et * P : (et + 1) * P],
                            rhs=xT[:, j, chn * 512 : (chn + 1) * 512],
                            start=(j == 0),
                            stop=(j == CT - 1),
                        )
                for et, chn in qgroups:
                    nc.vector.tensor_copy(
                        qT[:, et, chn * 512 : (chn + 1) * 512], pqs[(et, chn)]
                    )

            # ---------------- pass 0 with V/K projections inline -----------
            with (
                tc.tile_pool(name="sps0", bufs=1, space="PSUM") as sps0,
                tc.tile_pool(name="vps", bufs=1, space="PSUM") as vps,
            ):

                def emit_kproj(et, chm):
                    # shares the score-tile single-bank rotation (tag "st");
                    # eviction rides the otherwise-idle gpsimd engine
                    pk = sps0.tile([P, 512], F32, tag="st", name="pk", bufs=3)
                    for j in range(CT):
                        nc.tensor.matmul(
                            pk,
                            lhsT=wkT[:, j, et * P : (et + 1) * P],
                            rhs=ctxT[:, j, chm * 512 : (chm + 1) * 512],
                            start=(j == 0),
                            stop=(j == CT - 1),
                        )
                    nc.vector.tensor_scalar_add(
                        kT[:, et, chm * 512 : (chm + 1) * 512],
                        pk,
                        bk_sb[:, et : et + 1],
                    )

                def emit_kproj0():
                    # both et groups of chunk 0, contraction-chunk outer
                    pks = [sps0.tile([P, 512], F32, tag="st", name="pk", bufs=3)
                           for _ in range(2)]
                    for j in range(CT):
                        for et in range(2):
                            nc.tensor.matmul(
                                pks[et],
                                lhsT=wkT[:, j, et * P : (et + 1) * P],
                                rhs=ctxT[:, j, :512],
                                start=(j == 0),
                                stop=(j == CT - 1),
                            )
                    for et in range(2):
                        nc.vector.tensor_scalar_add(
                            kT[:, et, :512], pks[et], bk_sb[:, et : et + 1]
                        )

                def emit_vproj(mt):
                    pv = vps.tile([P, E], F32, tag="pv", name="pv")
                    for j in range(CT):
                        nc.tensor.matmul(
                            pv,
                            lhsT=ctxT[:, j, mt * P : (mt + 1) * P],
                            rhs=wvT[:, j, :],
                            start=(j == 0),
                            stop=(j == CT - 1),
                        )
                    nc.vector.tensor_copy(
                        vaug[:, mt, :, :HEAD_DIM],
                        _group_heads(pv[:, :], HPC, HEAD_DIM),
                    )

                emit_kproj0()
                emit_vproj(0)

                kfill = {2: (0, 1), 3: (1, 1), 6: (0, 2), 7: (1, 2),
                         10: (0, 3), 11: (1, 3)}
                with tc.tile_pool(name="ops0", bufs=1, space="PSUM") as ops0:
                    ot_ps0 = [
                        ops0.tile([HEAD_DIM + 1, 512], F32, tag=f"o{h}", name=f"o{h}")
                        for h in range(HPC)
                    ]
                    for mt in range(MT):
                        step = emit_step(sps0, ot_ps0, mt, 0)
                        next(step)
                        if mt < MT - 1:
                            emit_vproj(mt + 1)
                        next(step, None)
                        if 2 <= mt <= 5:
                            emit_ctx_q3(mt - 2)
                        if mt in kfill:
                            emit_kproj(*kfill[mt])
                    emit_pv(ot_ps0, MT - 2, 0)
                    emit_pv(ot_ps0, MT - 1, 0)
                    for h in range(HPC):
                        nc.vector.tensor_copy(ot_sb[:, h, :512], ot_ps0[h])

        def normalize_all(chn, srcs):
            """Normalize all 4 heads' n-half chn. srcs[h] rows 0:64 hold the
            unnormalized outputs (SBUF park or PSUM accumulators); the
            denominators must already sit in ot_sb row 64. One strided
            SBUF-SBUF DMA + one reciprocal + one partition_broadcast cover
            all heads (the broadcast is per-instruction-overhead bound).
            Returns {h: tmp} for the odd heads, for normalize_shift."""
            cs = slice(chn * 512, (chn + 1) * 512)
            dna = dnp.tile([1, HPC, 512], F32, tag="dna", name="dna")
            nc.sync.dma_start(out=dna, in_=ot_sb[HEAD_DIM : HEAD_DIM + 1, :, cs])
            rca = rbp.tile([1, HPC, 512], F32, tag="rca", name="rca")
            nc.vector.reciprocal_approx_fast(out=rca, in_=dna)
            rba = rbp.tile([HEAD_DIM, HPC, 512], F32, tag="rba", name="rba")
            nc.gpsimd.partition_broadcast(rba, rca)
            tmps = {}
            for h in range(HPC):
                hp, hl = divmod(h, 2)
                if hl == 0:
                    nc.vector.tensor_mul(otn2[:HEAD_DIM, hp, cs], srcs[h], rba[:, h, :])
                else:
                    tmp = rbp.tile([HEAD_DIM, 512], BF16, tag=f"tmp{h}", name="tmp")
                    nc.vector.tensor_mul(tmp, srcs[h], rba[:, h, :])
                    tmps[h] = tmp
            return tmps

        def normalize_shift(h, chn, tmp, yps):
            """odd-head normalized out -> partitions 64:128 via identity mm."""
            cs = slice(chn * 512, (chn + 1) * 512)
            hp = h // 2
            sh = yps.tile([P, 512], F32, tag="yp", name="sh")
            nc.tensor.matmul(
                sh[HEAD_DIM:P, :], lhsT=eye64, rhs=tmp, start=True, stop=True
            )
            nc.vector.tensor_copy(otn2[HEAD_DIM:P, hp, cs], sh[HEAD_DIM:P, :])

        def emit_oproj(yps, nb, ring):
            ys = ypool.tile([P, N], BF16, tag="ys", name="ys")
            for oc in range(DIM // 512):
                yp = yps.tile([P, 512], F32, tag="yp", name="yp")
                for hp in range(E // P):
                    nc.tensor.matmul(
                        yp,
                        lhsT=otn2[:, hp, nb * P : (nb + 1) * P],
                        rhs=woT[:, hp, oc * 512 : (oc + 1) * 512],
                        start=(hp == 0),
                        stop=(hp == E // P - 1),
                    )
                nc.vector.tensor_copy(ys[:, oc * 512 : (oc + 1) * 512], yp)
            ring.dma_start(out=y_d[nb * P : (nb + 1) * P, :], in_=ys)

        # ---------------- pass 1: n-cols 512:1024 + pass-0 tail work -------
        with tc.tile_pool(name="ops1", bufs=1, space="PSUM") as ops1:
            ot_ps1 = [
                ops1.tile([HEAD_DIM + 1, 512], F32, tag=f"p{h}", name=f"p{h}")
                for h in range(HPC)
            ]
            with (
                tc.tile_pool(name="sps1", bufs=1, space="PSUM") as sps1,
                tc.tile_pool(name="yps0", bufs=1, space="PSUM") as yps0,
            ):
                tmps = {}
                for mt in range(MT):
                    step = emit_step(sps1, ot_ps1, mt, 1)
                    next(step)
                    if mt in (4, 6, 8, 10):
                        emit_oproj(yps0, (mt - 4) // 2, nc.sync)
                    next(step, None)
                    if mt == 0:
                        tmps = normalize_all(
                            0, [ot_sb[:HEAD_DIM, h, :512] for h in range(HPC)]
                        )
                    elif mt == 2:
                        for h in (1, 3):
                            normalize_shift(h, 0, tmps[h], yps0)
                emit_pv(ot_ps1, MT - 2, 1)
                emit_pv(ot_ps1, MT - 1, 1)
                # stage pass-1 denominators on partition 64 for the tail
                for h in range(HPC):
                    nc.scalar.copy(
                        ot_sb[HEAD_DIM : HEAD_DIM + 1, h, 512:],
                        ot_ps1[h][HEAD_DIM : HEAD_DIM + 1, :],
                    )

            # tail: normalize straight from PSUM (scores pool closed -> 3
            # free banks), finish O proj, full-row y writeback on 3 rings
            with tc.tile_pool(name="yps1", bufs=3, space="PSUM") as yps1:
                tmps = normalize_all(1, [ot_ps1[h][:HEAD_DIM, :] for h in range(HPC)])
                for h in (1, 3):
                    normalize_shift(h, 1, tmps[h], yps1)
                rings = [nc.sync, nc.scalar, nc.gpsimd, nc.scalar]
                for i, nb in enumerate(range(N // P // 2, N // P)):
                    emit_oproj(yps1, nb, rings[i])

    nc.compile()
    return nc


_NC_CACHE = []


def _get_nc():
    if not _NC_CACHE:
        _NC_CACHE.append(build_program())
    return _NC_CACHE[0]


def make_in_maps(x, context, mask, Wq, Wkv, b_kv, Wo):
    bf = ml_dtypes.bfloat16
    f8 = ml_dtypes.float8_e4m3
    x = np.asarray(x, dtype=np.float32)
    context = np.asarray(context, dtype=np.float32)
    mask = np.asarray(mask)
    Wq = np.asarray(Wq, dtype=np.float32)
    Wkv = np.asarray(Wkv, dtype=np.float32)
    b_kv = np.asarray(b_kv, dtype=np.float32)
    Wo = np.asarray(Wo, dtype=np.float32)
    eye = np.eye(HEAD_DIM, dtype=bf)

    in_maps = []
    for b in range(B):
        xtb = np.ascontiguousarray(x[b].T).astype(f8)
        ctb = np.ascontiguousarray(context[b].T).astype(bf)
        mkb = np.ascontiguousarray(mask[b].T).astype(bf)
        for g in range(NUM_HEADS // HPC):
            sl = slice(E * g, E * (g + 1))
            in_maps.append(
                {
                    "xT": xtb,
                    "ctxT": ctb,
                    "mk": mkb,
                    "wqT": np.ascontiguousarray(Wq[sl].T).astype(bf),
                    "wkT": np.ascontiguousarray(Wkv[sl].T).astype(bf),
                    "wvT": np.ascontiguousarray(
                        Wkv[DIM + E * g : DIM + E * (g + 1)].T
                    ).astype(bf),
                    "woT": np.ascontiguousarray(Wo[:, sl].T).astype(bf),
                    "bk": np.ascontiguousarray(b_kv[sl]),
                    "eye64": eye,
                }
            )
    return in_maps


def combine_outputs(ys, b_kv, Wo):
    """ys: list of 8 per-core partial outputs [N, DIM], core order (b, g)."""
    b_v = np.asarray(b_kv, dtype=np.float32)[DIM:]
    ybias = np.asarray(Wo, dtype=np.float32) @ b_v  # [DIM]
    out = np.empty((B, N, DIM), dtype=np.float32)
    G = NUM_HEADS // HPC
    for b in range(B):
        acc = np.asarray(ys[G * b], dtype=np.float32)
        for g in range(1, G):
            acc = acc + np.asarray(ys[G * b + g], dtype=np.float32)
        out[b] = acc + ybias[None, :]
    return out


def kernel(x, context, mask, Wq, Wkv, b_kv, Wo):
    nc = _get_nc()
    in_maps = make_in_maps(x, context, mask, Wq, Wkv, b_kv, Wo)
    res = run_bass_kernel_spmd(nc, in_maps, core_ids=list(range(8)))
    ys = [m["y"] for m in res.results]
    return combine_outputs(ys, b_kv, Wo)


# revision 27
# speedup vs baseline: 1.0962x; 1.0101x over previous
"""CrossAttention Trainium2 kernel (8 NeuronCores, SPMD), bf16 compute.

Sharding: data-parallel over batch B=2, tensor-parallel over the 16 heads in
4 groups of 4 heads -> 8 cores, one (batch, head-group) pair each. Each core
computes its 4 heads' Q/K/V projections, masked softmax cross-attention, and
its partial output projection y_g = softmax(q k^T * scale) v @ Wo[:, g].T.
The host sums the 4 partial outputs per batch (the Wo row-split all-reduce,
done at unshard time) and adds the v-bias term Wo @ b_v, which is constant
across rows and factors out of the attention (softmax rows sum to 1).

Numerics: bf16 matmuls with fp32 PSUM accumulation; softmax statistics
(denominator, reciprocal, normalization) stay fp32. x travels as fp8e4m3
(halves its DMA footprint; the quantization only perturbs q and thus the
softmax weights, damped by the small score scale). y is written bf16 and
accumulated in fp32 on the host.

Layout: the PE contracts over the partition dim, so activations and weights
are laid out contraction-major. The host passes x/context/weight shards
already transposed (contraction axis leading) so every device DMA is a plain
contiguous row load; there are no transposes anywhere on the device.

Attention is computed scores-transposed: ST[m, n] per head, so the PV matmul
contracts over m directly. The softmax denominator comes for free from an
appended ones-column on the v stationary operand. exp() is unnormalized (no
max subtraction; scores*scale are bounded, |s| < ~4); mask zeros are applied
multiplicatively after exp.

Phase-B staging keeps the PE dense (HAM-warm) and overlaps the ACT-bound
exp stream with PE work:
  stage 1: scores+exp+mask for heads 0,1 (ACT-bound) with the V projection
           interleaved on the otherwise idle PE; masked exps parked in SBUF.
  stage 2: PV accumulation for heads 0,1 (dense PE) interleaved with
           scores+exp+mask for heads 2,3.
  stage 3: PV accumulation for heads 2,3, overlapped with the softmax
           normalization of heads 0,1.
"""

import numpy as np
import ml_dtypes

import concourse.bass as bass
import concourse.bacc as bacc
import concourse.mybir as mybir
import concourse.tile as tile
from concourse.bass_utils import run_bass_kernel_spmd

DIM = 1024
HEAD_DIM = 64
NUM_HEADS = 16
SCALE = HEAD_DIM**-0.5
B, N, M = 2, 1024, 2048
HPC = 4  # heads per core
E = HPC * HEAD_DIM  # 256: per-core projection width
P = 128
F32 = mybir.dt.float32
BF16 = mybir.dt.bfloat16
F8 = mybir.dt.float8e4
CT = DIM // P  # 8 contraction tiles
MT = M // P  # 16 m tiles


def _bc_heads(ap):
    """Broadcast a [P, N] AP to [P, 2, N] with a zero-stride head dim."""
    return bass.AP(ap.tensor, ap.offset, [ap.ap[0], [0, 2], ap.ap[1]])


def build_program():
    nc = bacc.Bacc("TRN2", target_bir_lowering=False, debug=False, num_devices=8)

    # all activation/weight shards arrive contraction-major (pre-transposed)
    xT_d = nc.dram_tensor("xT", [DIM, N], F8, kind="ExternalInput").ap()
    ctxT_d = nc.dram_tensor("ctxT", [DIM, M], BF16, kind="ExternalInput").ap()
    maskt_d = nc.dram_tensor("maskt", [M, N], BF16, kind="ExternalInput").ap()
    wqT_d = nc.dram_tensor("wqT", [DIM, E], BF16, kind="ExternalInput").ap()
    wkT_d = nc.dram_tensor("wkT", [DIM, E], BF16, kind="ExternalInput").ap()
    wvT_d = nc.dram_tensor("wvT", [DIM, E], BF16, kind="ExternalInput").ap()
    woT_d = nc.dram_tensor("woT", [E, DIM], BF16, kind="ExternalInput").ap()
    bk_d = nc.dram_tensor("bk", [E], F32, kind="ExternalInput").ap()
    y_d = nc.dram_tensor("y", [N, DIM], BF16, kind="ExternalOutput").ap()

    Exp = mybir.ActivationFunctionType.Exp

    from contextlib import ExitStack

    with tile.TileContext(nc) as tc, ExitStack() as ctx:
        const = ctx.enter_context(tc.tile_pool(name="const", bufs=1))
        bk_sb = const.tile([P, E // P], F32)
        nc.sync.dma_start(out=bk_sb, in_=bk_d.rearrange("(t p) -> p t", p=P))

        persist = ctx.enter_context(tc.tile_pool(name="persist", bufs=1))
        qT = persist.tile([P, E // P, N], BF16)
        kT = persist.tile([P, E // P, M], BF16)
        vaug = persist.tile([P, MT, HPC, HEAD_DIM + 1], BF16)
        woT = persist.tile([P, E // P, DIM], BF16)
        # rows 0:64 unnormalized attention out, row 64 denominator
        ot_sb = persist.tile([HEAD_DIM + 1, HPC, N], F32)
        otn2 = persist.tile([P, E // P, N], BF16)

        # ones column: fill everything; v evictions overwrite cols 0:64
        nc.vector.memset(vaug, 1.0)

        bwork = ctx.enter_context(tc.tile_pool(name="bwork", bufs=4))
        maskp = ctx.enter_context(tc.tile_pool(name="maskp", bufs=3))
        rbp = ctx.enter_context(tc.tile_pool(name="rbp", bufs=2))

        def emit_scores(spool, sbufs, hp, mt, exmst, mk):
            """scores -> exp -> mask for head pair hp at m-tile mt,
            per n-chunk PSUM tiles so the next tile can double-buffer."""
            for chn in range(N // 512):
                st = spool.tile(
                    [P, 2, 512], F32, tag="st", name="st", bufs=sbufs
                )
                for hl in range(2):
                    erow = slice(hl * HEAD_DIM, (hl + 1) * HEAD_DIM)
                    nc.tensor.matmul(
                        st[:, hl, :],
                        lhsT=kT[erow, hp, mt * P : (mt + 1) * P],
                        rhs=qT[erow, hp, chn * 512 : (chn + 1) * 512],
                        start=True,
                        stop=True,
                    )
                ex = bwork.tile([P, 2, 512], BF16, tag="ex", name="ex")
                nc.scalar.activation(ex, st, Exp, scale=float(SCALE))
                mks = mk[:, chn * 512 : (chn + 1) * 512]
                mkc = bass.AP(mks.tensor, mks.offset, [mks.ap[0], [0, 2], mks.ap[1]])
                nc.vector.tensor_mul(
                    exmst[:, mt, :, chn * 512 : (chn + 1) * 512], ex, mkc
                )

        def emit_pv(ot_ps, hp, mt, exmst):
            for hl in range(2):
                h = hp * 2 + hl
                for chn in range(N // 512):
                    nc.tensor.matmul(
                        ot_ps[hl * 2 + chn],
                        lhsT=vaug[:, mt, h, :],
                        rhs=exmst[:, mt, hl, chn * 512 : (chn + 1) * 512],
                        start=(mt == 0),
                        stop=(mt == MT - 1),
                    )

        def evict_ot(ot_ps, hp):
            for hl in range(2):
                for chn in range(2):
                    nc.vector.tensor_copy(
                        ot_sb[:, hp * 2 + hl, chn * 512 : (chn + 1) * 512],
                        ot_ps[hl * 2 + chn],
                    )

        def normalize_head(h, dn_pool):
            """softmax-normalize head h into its otn2 half."""
            hp, hl = divmod(h, 2)
            dn = slice(HEAD_DIM, HEAD_DIM + 1)
            # partition_broadcast only reads partition 0 on HW: move the
            # denominator row (partition 64) to partition 0 via SBUF DMA.
            dn_sb = dn_pool.tile([1, N], F32, tag="dn", name="dn", bufs=2)
            nc.sync.dma_start(out=dn_sb, in_=ot_sb[dn, h, :])
            rbr = rbp.tile([HEAD_DIM, N], F32, tag="rbr", name="rbr")
            nc.gpsimd.partition_broadcast(rbr, dn_sb[0:1, :])
            rb = rbp.tile([HEAD_DIM, N], F32, tag="rb", name="rb")
            nc.vector.reciprocal_approx_fast(out=rb, in_=rbr)
            if hl == 0:
                nc.vector.tensor_mul(
                    otn2[:HEAD_DIM, hp, :], ot_sb[:HEAD_DIM, h, :], rb
                )
            else:
                tmp = rbp.tile([HEAD_DIM, N], BF16, tag="tmp", name="tmp")
                nc.vector.tensor_mul(tmp, ot_sb[:HEAD_DIM, h, :], rb)
                # partition shift 0:64 -> 64:128 via SBUF-SBUF DMA
                nc.sync.dma_start(out=otn2[HEAD_DIM:P, hp, :], in_=tmp)

        def load_mask(mt):
            mk = maskp.tile([P, N], BF16, tag="mk", name="mk")
            nc.gpsimd.dma_start(out=mk, in_=maskt_d[mt * P : (mt + 1) * P, :])
            return mk

        with tc.tile_pool(name="exmp", bufs=1) as exmp:
            # masked exp(scores) parked per m-tile; one buffer reused across
            # head pairs (WAR: stage-2 rewrites a tile only after its PV read)
            exmst = exmp.tile([P, MT, 2, N], BF16)

            with tc.tile_pool(name="wctx", bufs=1) as wctx_pool:
                wkT = wctx_pool.tile([P, CT, E], BF16)
                wvT = wctx_pool.tile([P, CT, E], BF16)
                ctxT = wctx_pool.tile([P, CT, M], BF16)

                with tc.tile_pool(name="qx", bufs=1) as qx_pool:
                    wqT = qx_pool.tile([P, CT, E], BF16)
                    xT = qx_pool.tile([P, CT, N], F8)
                    # plain contiguous loads, dependency-first order; ctx/k/v
                    # weights ride the second HWDGE ring (scalar) in parallel
                    for j in range(CT):
                        nc.sync.dma_start(
                            out=wqT[:, j, :], in_=wqT_d[j * P : (j + 1) * P, :]
                        )
                        nc.sync.dma_start(
                            out=xT[:, j, :], in_=xT_d[j * P : (j + 1) * P, :]
                        )
                    for j in range(CT):
                        nc.scalar.dma_start(
                            out=wkT[:, j, :], in_=wkT_d[j * P : (j + 1) * P, :]
                        )
                        nc.scalar.dma_start(
                            out=ctxT[:, j, :], in_=ctxT_d[j * P : (j + 1) * P, :]
                        )
                    for j in range(CT):
                        nc.scalar.dma_start(
                            out=wvT[:, j, :], in_=wvT_d[j * P : (j + 1) * P, :]
                        )
                    for t in range(E // P):
                        nc.scalar.dma_start(
                            out=woT[:, t, :], in_=woT_d[t * P : (t + 1) * P, :]
                        )

                    # Q projection
                    with tc.tile_pool(name="ppsA", bufs=3, space="PSUM") as ppsA:
                        for et in range(E // P):
                            for chn in range(N // 512):
                                pq = ppsA.tile([P, 512], F32, tag="pq")
                                for j in range(CT):
                                    nc.tensor.matmul(
                                        pq,
                                        lhsT=wqT[:, j, et * P : (et + 1) * P],
                                        rhs=xT[:, j, chn * 512 : (chn + 1) * 512],
                                        start=(j == 0),
                                        stop=(j == CT - 1),
                                    )
                                nc.vector.tensor_copy(
                                    qT[:, et, chn * 512 : (chn + 1) * 512], pq
                                )

                def emit_kproj(kps, et, chm):
                    pk = kps.tile([P, 512], F32, tag="pk", name="pk")
                    for j in range(CT):
                        nc.tensor.matmul(
                            pk,
                            lhsT=wkT[:, j, et * P : (et + 1) * P],
                            rhs=ctxT[:, j, chm * 512 : (chm + 1) * 512],
                            start=(j == 0),
                            stop=(j == CT - 1),
                        )
                    nc.vector.tensor_scalar_add(
                        kT[:, et, chm * 512 : (chm + 1) * 512],
                        pk,
                        bk_sb[:, et : et + 1],
                    )

                # K projection for the first head pair's first chunk must
                # precede stage 1; the rest is folded into stage 1's PE slack.
                # stage 1: scores(heads 0,1) [ACT-bound] + V and K
                # projections interleaved on the otherwise idle PE.
                with (
                    tc.tile_pool(name="sps1", bufs=1, space="PSUM") as sps1,
                    tc.tile_pool(name="vps", bufs=2, space="PSUM") as vps,
                    tc.tile_pool(name="kps", bufs=2, space="PSUM") as kps,
                ):
                    emit_kproj(kps, 0, 0)
                    for mt in range(MT):
                        # keep kT(et0) one chunk ahead of the scores that
                        # consume it; kT(et1) lands before stage 2
                        if mt % 2 == 0:
                            et, chm = divmod(mt // 2 + 1, M // 512)
                            if et < 2:
                                emit_kproj(kps, et, chm)
                        mk = load_mask(mt)
                        emit_scores(sps1, 2, 0, mt, exmst, mk)
                        pv = vps.tile([P, E], F32, tag="pv")
                        for j in range(CT):
                            nc.tensor.matmul(
                                pv,
                                lhsT=ctxT[:, j, mt * P : (mt + 1) * P],
                                rhs=wvT[:, j, :],
                                start=(j == 0),
                                stop=(j == CT - 1),
                            )
                        for h in range(HPC):
                            nc.vector.tensor_copy(
                                vaug[:, mt, h, :HEAD_DIM],
                                pv[:, h * HEAD_DIM : (h + 1) * HEAD_DIM],
                            )

            # stage 2: PV(heads 0,1) interleaved with scores(heads 2,3)
            with tc.tile_pool(name="ops0", bufs=1, space="PSUM") as ops0:
                ot_ps0 = [
                    ops0.tile([HEAD_DIM + 1, 512], F32, tag=f"o{i}", name=f"o{i}")
                    for i in range(4)
                ]
                with tc.tile_pool(name="sps2", bufs=1, space="PSUM") as sps2:
                    for mt in range(MT):
                        mk = load_mask(mt)
                        emit_pv(ot_ps0, 0, mt, exmst)
                        emit_scores(sps2, 2, 1, mt, exmst, mk)
                evict_ot(ot_ps0, 0)

            # stage 3: PV per head (2 then 3); normalization of earlier heads
            # overlaps the remaining PV sweeps
            with (
                tc.tile_pool(name="ops1", bufs=1, space="PSUM") as ops1,
                tc.tile_pool(name="dnp", bufs=1) as dnp,
            ):
                ot_ps1 = [
                    ops1.tile([HEAD_DIM + 1, 512], F32, tag=f"p{i}", name=f"p{i}")
                    for i in range(4)
                ]
                normalize_head(0, dnp)
                normalize_head(1, dnp)
                for mt in range(MT):
                    emit_pv(ot_ps1, 1, mt, exmst)
                evict_ot(ot_ps1, 1)
                normalize_head(2, dnp)
                normalize_head(3, dnp)

        # ---------- output projection ----------
        with (
            tc.tile_pool(name="ypsum", bufs=3, space="PSUM") as ypsum,
            tc.tile_pool(name="ypool", bufs=3) as ypool,
        ):
            for nb in range(N // P):
                for oc in range(DIM // 512):
                    yp = ypsum.tile([P, 512], F32, tag="yp")
                    for hp in range(E // P):
                        nc.tensor.matmul(
                            yp,
                            lhsT=otn2[:, hp, nb * P : (nb + 1) * P],
                            rhs=woT[:, hp, oc * 512 : (oc + 1) * 512],
                            start=(hp == 0),
                            stop=(hp == E // P - 1),
                        )
                    ys = ypool.tile([P, 512], BF16, tag="ys")
                    nc.vector.tensor_copy(ys, yp)
                    ring = nc.scalar if (nb + oc) % 2 else nc.sync
                    ring.dma_start(
                        out=y_d[nb * P : (nb + 1) * P, oc * 512 : (oc + 1) * 512],
                        in_=ys,
                    )

    nc.compile()
    return nc


_NC_CACHE = []


def _get_nc():
    if not _NC_CACHE:
        _NC_CACHE.append(build_program())
    return _NC_CACHE[0]


def make_in_maps(x, context, mask, Wq, Wkv, b_kv, Wo):
    bf = ml_dtypes.bfloat16
    f8 = ml_dtypes.float8_e4m3
    x = np.asarray(x, dtype=np.float32)
    context = np.asarray(context, dtype=np.float32)
    mask = np.asarray(mask)
    Wq = np.asarray(Wq, dtype=np.float32)
    Wkv = np.asarray(Wkv, dtype=np.float32)
    b_kv = np.asarray(b_kv, dtype=np.float32)
    Wo = np.asarray(Wo, dtype=np.float32)

    in_maps = []
    for b in range(B):
        xtb = np.ascontiguousarray(x[b].T).astype(f8)
        ctb = np.ascontiguousarray(context[b].T).astype(bf)
        mtb = np.ascontiguousarray(mask[b].T).astype(bf)
        for g in range(NUM_HEADS // HPC):
            sl = slice(E * g, E * (g + 1))
            in_maps.append(
                {
                    "xT": xtb,
                    "ctxT": ctb,
                    "maskt": mtb,
                    "wqT": np.ascontiguousarray(Wq[sl].T).astype(bf),
                    "wkT": np.ascontiguousarray(Wkv[sl].T).astype(bf),
                    "wvT": np.ascontiguousarray(
                        Wkv[DIM + E * g : DIM + E * (g + 1)].T
                    ).astype(bf),
                    "woT": np.ascontiguousarray(Wo[:, sl].T).astype(bf),
                    "bk": np.ascontiguousarray(b_kv[sl]),
                }
            )
    return in_maps


def combine_outputs(ys, b_kv, Wo):
    """ys: list of 8 per-core partial outputs [N, DIM], core order (b, g)."""
    b_v = np.asarray(b_kv, dtype=np.float32)[DIM:]
    ybias = np.asarray(Wo, dtype=np.float32) @ b_v  # [DIM]
    out = np.empty((B, N, DIM), dtype=np.float32)
    G = NUM_HEADS // HPC
    for b in range(B):
        acc = np.asarray(ys[G * b], dtype=np.float32)
        for g in range(1, G):
            acc = acc + np.asarray(ys[G * b + g], dtype=np.float32)
        out[b] = acc + ybias[None, :]
    return out


def kernel(x, context, mask, Wq, Wkv, b_kv, Wo):
    nc = _get_nc()
    in_maps = make_in_maps(x, context, mask, Wq, Wkv, b_kv, Wo)
    res = run_bass_kernel_spmd(nc, in_maps, core_ids=list(range(8)))
    ys = [m["y"] for m in res.results]
    return combine_outputs(ys, b_kv, Wo)
